# revision 7
# baseline (speedup 1.0000x reference)
"""Cross-attention Trainium2 Bass kernel, 8-core SPMD.

Problem (hardcoded): B=2, SQ=SKV=2048, DIM=1024, H=16, hd=64, fp32 I/O.

Sharding: 8 cores = 8 head-pairs (2 heads / 128 cols each); every core
processes BOTH batches. Tensor-parallel on wq/wk/wv columns and wo rows.
Unlike the earlier 2x4 layout this ships each weight slice exactly once
and never duplicates activations: each core uploads a distinct 1/8 slice
of X=[xh0;xh1;xc0;xc1] ([8192,1024] bf16) and an on-device AllGather
rebuilds the full X. The wo-row all-reduce is an on-device bf16
ReduceScatter over all 8 cores, so each core downloads only its
[512,1024] slice of the final output (bf16); the host just concatenates
and adds bo. This cuts host<->device traffic ~5x vs shipping full
duplicated inputs and full fp32 partial outputs, which dominates
wall-clock through the axon tunnel.

Per-core dataflow (all matmuls bf16 with fp32 PSUM accumulation), per
batch b in {0,1}:
  1. DMA-transpose the bf16 inputs into x^T layout [128d, dc, S] straight
     from the gathered X in DRAM (XBAR path), then project to Q^T/K^T in
     head-on-partition layout [128c, S] (+bias per partition) and V in
     natural layout [128k, kc, h, 65] with a ones column appended so the
     PV matmul also produces softmax denominators.
  2. Flash-style attention per q-tile of 128: scores^T chunks [k128,
     q128] accumulate in PSUM (8 k-chunks per half), one exp on ScalarE
     per (head, half) (PSUM fp32 -> SBUF bf16, scale=1/8 folded in),
     then PV with P^T stationary: psum_o[q, 65] += P^T.T@V'. The two
     heads sit in partition rows 0-63/64-127 so the PE 64-row array
     tiling can run both heads' matmuls concurrently.
  3. Normalize O by the per-partition reciprocal of column 64, small PE
     transpose to O^T, then the wo projection out[q,1024] per q-tile
     into the [4096,1024] f32 partial that feeds the ReduceScatter.
Softmax skips the max-subtraction: scores here are ~N(0,1) (max |s| ~ 8),
so exp stays comfortably in fp32/bf16 range and matches jax softmax to
fp32 noise.
"""

import numpy as np
import ml_dtypes

try:  # persistent jit cache: skips ~1s/call of XLA->walrus recompile
    import jax

    jax.config.update("jax_compilation_cache_dir", "/tmp/jax_cc_cache")
    jax.config.update("jax_persistent_cache_min_entry_size_bytes", 0)
    jax.config.update("jax_persistent_cache_min_compile_time_secs", 0.0)
except Exception:
    pass

import concourse.bass as bass
import concourse.mybir as mybir
import concourse.tile as tile
from concourse.masks import make_identity

F32 = mybir.dt.float32
BF16 = mybir.dt.bfloat16
AF = mybir.ActivationFunctionType

S = 2048  # sequence length (both q and kv)
D = 1024  # model dim
NB = 2  # batches
HL = 2  # local heads per core
HD = 64  # head dim
CS = HL * HD  # local channel slice = 128
P = 128
DC = D // P  # 8 d-chunks
NSC = S // P  # 16 s-chunks
NQT = S // P  # 16 q-tiles
XROWS = NB * 2 * S  # 8192 rows of X = [xh0; xh1; xc0; xc1]
XSL = XROWS // 8  # 1024 rows shipped per core
OROWS = NB * S  # 4096 rows of stacked partial output
OSL = OROWS // 8  # 512 rows of final output per core

GROUPS = [list(range(8))]


def build_program(iters=1):
    nc = bass.Bass("TRN2", target_bir_lowering=False, debug=False, num_devices=8)

    xin = nc.dram_tensor("xin", [XSL, D], BF16, kind="ExternalInput").ap()
    wq = nc.dram_tensor("wq", [D, CS], BF16, kind="ExternalInput").ap()
    wk = nc.dram_tensor("wk", [D, CS], BF16, kind="ExternalInput").ap()
    wv = nc.dram_tensor("wv", [D, CS], BF16, kind="ExternalInput").ap()
    wo = nc.dram_tensor("wo", [CS, D], BF16, kind="ExternalInput").ap()
    bq = nc.dram_tensor("bq", [CS], F32, kind="ExternalInput").ap()
    bk = nc.dram_tensor("bk", [CS], F32, kind="ExternalInput").ap()
    bv = nc.dram_tensor("bv", [CS], F32, kind="ExternalInput").ap()
    out_s = nc.dram_tensor("out_s", [OSL, D], BF16, kind="ExternalOutput").ap()

    with tile.TileContext(nc) as tc:
        with (
            tc.tile_pool(name="dram", bufs=1, space="DRAM") as dram,
            tc.tile_pool(name="persist", bufs=1) as pp,
            tc.tile_pool(name="ptp", bufs=3) as ptp,
            tc.tile_pool(name="osb", bufs=2) as osb,
            tc.tile_pool(name="outp", bufs=2) as outp,
            tc.tile_pool(name="smalls", bufs=2) as smalls,
            tc.tile_pool(name="ps", bufs=3, space="PSUM") as ps,
        ):
            # ---- DRAM bounces for the collectives ----
            x_bounce = dram.tile([XSL, D], BF16)
            Xg = dram.tile([XROWS, D], BF16)
            out_part = dram.tile([OROWS, D], BF16)
            out_red = dram.tile([OSL, D], BF16)

            # DRAM->DRAM DMA is not safe in this environment (NRT exec-unit
            # crash); bounce the input through SBUF instead.
            for i in range(XSL // P):
                xtmp = outp.tile([P, D], BF16, tag="xtmp", name="xtmp")
                nc.sync.dma_start(xtmp, xin[i * P : (i + 1) * P, :])
                nc.sync.dma_start(x_bounce[i * P : (i + 1) * P, :], xtmp)
            nc.gpsimd.collective_compute(
                "AllGather",
                mybir.AluOpType.bypass,
                replica_groups=GROUPS,
                ins=[x_bounce.opt()],
                outs=[Xg.opt()],
            )

            # ---- persistent SBUF state ----
            wq_sb = pp.tile([P, DC, CS], BF16)
            wk_sb = pp.tile([P, DC, CS], BF16)
            wv_sb = pp.tile([P, DC, CS], BF16)
            wo_sb = pp.tile([P, D], BF16)
            bqc = pp.tile([P, 1], F32)
            bkc = pp.tile([P, 1], F32)
            bv_row = pp.tile([1, CS], F32)
            ones1 = pp.tile([1, P], F32)
            bv_bc = pp.tile([P, CS], F32)
            ident = pp.tile([P, P], BF16)

            nc.sync.dma_start(wq_sb, wq.rearrange("(o p) c -> p o c", p=P))
            nc.sync.dma_start(wk_sb, wk.rearrange("(o p) c -> p o c", p=P))
            nc.sync.dma_start(wv_sb, wv.rearrange("(o p) c -> p o c", p=P))
            nc.sync.dma_start(wo_sb, wo)
            nc.sync.dma_start(bqc, bq[:, None])
            nc.sync.dma_start(bkc, bk[:, None])
            nc.sync.dma_start(bv_row, bv[None, :])
            make_identity(nc, ident)
            nc.vector.memset(ones1, 1.0)

            # broadcast bv across partitions: [1,128] -> [128,128] via fp32 matmul
            psb = ps.tile([P, CS], F32, tag="o", bufs=2, name="psb")
            nc.tensor.matmul(psb, ones1, bv_row, start=True, stop=True)
            nc.vector.tensor_copy(bv_bc, psb)

            def one_batch(b):
                xhT = pp.tile([P, DC, S], BF16, tag="xhT", name="xhT")
                xcT = pp.tile([P, DC, S], BF16, tag="xcT", name="xcT")
                QT = pp.tile([P, S], BF16, tag="QT", name="QT")
                KT = pp.tile([P, S], BF16, tag="KT", name="KT")
                Vp = pp.tile([P, NSC, HL, HD + 1], BF16, tag="Vp", name="Vp")
                OT = pp.tile([P, S], BF16, tag="OT", name="OT")
                # ones column for the PV sums trick; V writes only fill [..., 0:64]
                nc.vector.memset(Vp, 1.0)

                xh_rows = b * S  # rows of xh[b] in Xg
                xc_rows = 2 * S + b * S  # rows of xc[b] in Xg

                # ---- transposed input loads (XBAR dma transpose, bf16) ----
                SG = 512
                for g in range(S // SG):
                    sl = slice(g * SG, (g + 1) * SG)
                    gl = slice(xc_rows + g * SG, xc_rows + (g + 1) * SG)
                    for dc in range(DC):
                        nc.sync.dma_start_transpose(
                            xcT[:, dc, sl], Xg[gl, dc * P : (dc + 1) * P]
                        )
                for g in range(S // SG):
                    sl = slice(g * SG, (g + 1) * SG)
                    gl = slice(xh_rows + g * SG, xh_rows + (g + 1) * SG)
                    for dc in range(DC):
                        nc.sync.dma_start_transpose(
                            xhT[:, dc, sl], Xg[gl, dc * P : (dc + 1) * P]
                        )

                # ---- projections ----
                def proj_kv(sc):
                    ss = sc * P
                    psk = ps.tile([P, P], F32, tag="o", bufs=2, name="psk")
                    for dc in range(DC):
                        nc.tensor.matmul(
                            psk,
                            wk_sb[:, dc, :],
                            xcT[:, dc, ss : ss + P],
                            start=(dc == 0),
                            stop=(dc == DC - 1),
                        )
                    nc.vector.tensor_scalar_add(KT[:, ss : ss + P], psk, bkc)
                    psv = ps.tile([P, CS], F32, tag="o", bufs=2, name="psv")
                    for dc in range(DC):
                        nc.tensor.matmul(
                            psv,
                            xcT[:, dc, ss : ss + P],
                            wv_sb[:, dc, :],
                            start=(dc == 0),
                            stop=(dc == DC - 1),
                        )
                    for h in range(HL):
                        nc.vector.tensor_add(
                            Vp[:, sc, h, 0:HD],
                            psv[:, h * HD : (h + 1) * HD],
                            bv_bc[:, h * HD : (h + 1) * HD],
                        )

                def proj_q(sc):
                    ss = sc * P
                    psq = ps.tile([P, P], F32, tag="o", bufs=2, name="psq")
                    for dc in range(DC):
                        nc.tensor.matmul(
                            psq,
                            wq_sb[:, dc, :],
                            xhT[:, dc, ss : ss + P],
                            start=(dc == 0),
                            stop=(dc == DC - 1),
                        )
                    nc.vector.tensor_scalar_add(QT[:, ss : ss + P], psq, bqc)

                for sc in range(NSC):
                    proj_kv(sc)

                # ---- attention + output projection, per q-tile ----
                scale = 1.0 / float(np.sqrt(HD))
                NHALF = NSC // 2  # 8 k-chunks per half
                for qt in range(NQT):
                    qs = qt * P
                    proj_q(qt)
                    ps_o = [
                        ps.tile([P, HD + 1], F32, tag="o", bufs=2, name="ps_o")
                        for _ in range(2)
                    ]
                    for half in range(2):
                        psc = [
                            ps.tile([P, NHALF, P], F32, tag="s", name="psc")
                            for _ in range(2)
                        ]
                        for j in range(NHALF):
                            ks = (half * NHALF + j) * P
                            for hh in range(2):
                                e0 = hh * HD
                                nc.tensor.matmul(
                                    psc[hh][:, j, :],
                                    KT[e0 : e0 + HD, ks : ks + P],
                                    QT[e0 : e0 + HD, qs : qs + P],
                                    start=True,
                                    stop=True,
                                )
                        pt = [
                            ptp.tile([P, NHALF, P], BF16, tag="pt", name="pt")
                            for _ in range(2)
                        ]
                        for hh in range(2):
                            nc.scalar.activation(pt[hh], psc[hh], AF.Exp, scale=scale)
                        for j in range(NHALF):
                            kc = half * NHALF + j
                            for hh in range(2):
                                nc.tensor.matmul(
                                    ps_o[hh],
                                    pt[hh][:, j, :],
                                    Vp[:, kc, hh, :],
                                    start=(kc == 0),
                                    stop=(kc == NSC - 1),
                                )
                    recip = smalls.tile([P, 2], F32, tag="recip")
                    for hh in range(2):
                        nc.vector.reciprocal(
                            recip[:, hh : hh + 1], ps_o[hh][:, HD : HD + 1]
                        )
                    for hh in range(2):
                        e0 = hh * HD
                        o_sb = osb.tile([P, HD], BF16, tag="o_sb")
                        nc.vector.tensor_scalar_mul(
                            o_sb, ps_o[hh][:, 0:HD], recip[:, hh : hh + 1]
                        )
                        ps_ot = ps.tile([HD, P], BF16, tag="o", bufs=2, name="ps_ot")
                        nc.tensor.transpose(ps_ot, o_sb, ident)
                        nc.vector.tensor_copy(OT[e0 : e0 + HD, qs : qs + P], ps_ot)
                    out_sb = outp.tile([P, D], BF16, tag="out_sb")
                    for nj in range(2):
                        ps3 = ps.tile([P, D // 2], F32, tag="o", bufs=2, name="ps3")
                        nc.tensor.matmul(
                            ps3,
                            OT[:, qs : qs + P],
                            wo_sb[:, nj * (D // 2) : (nj + 1) * (D // 2)],
                            start=True,
                            stop=True,
                        )
                        nc.vector.tensor_copy(
                            out_sb[:, nj * (D // 2) : (nj + 1) * (D // 2)], ps3
                        )
                    nc.sync.dma_start(
                        out_part[b * S + qs : b * S + qs + P, :], out_sb
                    )

            def one_pass():
                for b in range(NB):
                    one_batch(b)

                # ---- on-device all-reduce of the wo partials (bf16 CCE add) ----
                nc.gpsimd.collective_compute(
                    "ReduceScatter",
                    mybir.AluOpType.add,
                    replica_groups=GROUPS,
                    ins=[out_part.opt()],
                    outs=[out_red.opt()],
                )
                # bounce DRAM->SBUF->DRAM (direct DRAM->DRAM DMA crashes NRT here)
                for i in range(OSL // P):
                    finb = outp.tile([P, D], BF16, tag="finb", name="finb")
                    nc.sync.dma_start(finb, out_red[i * P : (i + 1) * P, :])
                    nc.sync.dma_start(out_s[i * P : (i + 1) * P, :], finb)

            for _it in range(iters):
                one_pass()

    _split_matmul_waits(nc)
    return nc


_WAIT_EXEMPT = (
    mybir.InstAllEngineBarrier,
    mybir.InstEventSemaphore,
    mybir.InstNoOp,
)


def _split_matmul_waits(nc):
    """This walrus' ISA structs encode only ONE sync wait per instruction;
    Tile can emit 2+ (e.g. fresh-DMA input + psum-slot WAR). Hoist all but
    one wait onto engine no-ops inserted just before the instruction."""
    n_id = 0
    for blk in nc.m.functions[0].blocks:
        new_insts = []
        for inst in blk.instructions:
            si = getattr(inst, "sync_info", None)
            if (
                not isinstance(inst, _WAIT_EXEMPT)
                and si is not None
                and si.on_wait
                and len(si.on_wait) > 1
            ):
                waits = list(si.on_wait)
                for w in waits[:-1]:
                    n_id += 1
                    new_insts.append(
                        mybir.InstNoOp(
                            name=f"I-waitsplit-{n_id}",
                            engine=inst.engine,
                            sync_info=mybir.SyncInfo(on_wait=[w], on_update=[]),
                        )
                    )
                inst.sync_info = mybir.SyncInfo(
                    on_wait=[waits[-1]], on_update=list(si.on_update or [])
                )
            new_insts.append(inst)
        blk.instructions[:] = new_insts
    return nc


_CACHE = {}


def _get_program():
    if "nc" not in _CACHE:
        _CACHE["nc"] = build_program()
    return _CACHE["nc"]


def make_in_maps(inputs):
    bf16 = ml_dtypes.bfloat16
    hid = np.asarray(inputs["hidden_states"])
    ctx = np.asarray(inputs["context"])
    # X = [xh0; xh1; xc0; xc1] : [8192, 1024]
    X = np.concatenate([hid[0], hid[1], ctx[0], ctx[1]], axis=0).astype(bf16)
    wq_b = np.asarray(inputs["wq"]).astype(bf16)
    wk_b = np.asarray(inputs["wk"]).astype(bf16)
    wv_b = np.asarray(inputs["wv"]).astype(bf16)
    wo_b = np.asarray(inputs["wo"]).astype(bf16)
    in_maps = []
    for cid in range(8):
        cs = slice(cid * CS, (cid + 1) * CS)
        in_maps.append(
            {
                "xin": np.ascontiguousarray(X[cid * XSL : (cid + 1) * XSL]),
                "wq": np.ascontiguousarray(wq_b[:, cs]),
                "wk": np.ascontiguousarray(wk_b[:, cs]),
                "wv": np.ascontiguousarray(wv_b[:, cs]),
                "wo": np.ascontiguousarray(wo_b[cs, :]),
                "bq": np.ascontiguousarray(np.asarray(inputs["bq"])[cs]).astype(np.float32),
                "bk": np.ascontiguousarray(np.asarray(inputs["bk"])[cs]).astype(np.float32),
                "bv": np.ascontiguousarray(np.asarray(inputs["bv"])[cs]).astype(np.float32),
            }
        )
    return in_maps


def assemble(results, inputs):
    out = np.concatenate(
        [np.asarray(results[c]["out_s"], np.float32) for c in range(8)], axis=0
    ).reshape(NB, S, D)
    out += np.asarray(inputs["bo"], np.float32)[None, None, :]
    return out


def kernel(**inputs):
    from concourse.bass_utils import run_bass_kernel_spmd

    nc = _get_program()
    in_maps = make_in_maps(inputs)
    res = run_bass_kernel_spmd(nc, in_maps, list(range(8)))
    return assemble(res.results, inputs)


# revision 11
# speedup vs baseline: 1.0212x; 1.0212x over previous
"""Cross-attention Trainium2 Bass kernel, 8-core SPMD.

Problem (hardcoded): B=2, SQ=SKV=2048, DIM=1024, H=16, hd=64, fp32 I/O.

Sharding: 8 cores = 8 head-pairs (2 heads / 128 cols each); every core
processes BOTH batches. Tensor-parallel on wq/wk/wv columns and wo rows.
Unlike the earlier 2x4 layout this ships each weight slice exactly once
and never duplicates activations: each core uploads a distinct 1/8 slice
of X=[xh0;xh1;xc0;xc1] ([8192,1024] bf16) and an on-device AllGather
rebuilds the full X. The wo-row all-reduce is an on-device bf16
ReduceScatter over all 8 cores, so each core downloads only its
[512,1024] slice of the final output (bf16); the host just concatenates
and adds bo. This cuts host<->device traffic ~5x vs shipping full
duplicated inputs and full fp32 partial outputs, which dominates
wall-clock through the axon tunnel.

Per-core dataflow (all matmuls bf16 with fp32 PSUM accumulation), per
batch b in {0,1}:
  1. DMA-transpose the bf16 inputs into x^T layout [128d, dc, S] straight
     from the gathered X in DRAM (XBAR path), then project to Q^T/K^T in
     head-on-partition layout [128c, S] (+bias per partition) and V in
     natural layout [128k, kc, h, 65] with a ones column appended so the
     PV matmul also produces softmax denominators.
  2. Flash-style attention per q-tile of 128: scores^T chunks [k128,
     q128] accumulate in PSUM (8 k-chunks per half), one exp on ScalarE
     per (head, half) (PSUM fp32 -> SBUF bf16, scale=1/8 folded in),
     then PV with P^T stationary: psum_o[q, 65] += P^T.T@V'. The two
     heads sit in partition rows 0-63/64-127 so the PE 64-row array
     tiling can run both heads' matmuls concurrently.
  3. Normalize O by the per-partition reciprocal of column 64, small PE
     transpose to O^T, then the wo projection out[q,1024] per q-tile
     into the [4096,1024] f32 partial that feeds the ReduceScatter.
Softmax skips the max-subtraction: scores here are ~N(0,1) (max |s| ~ 8),
so exp stays comfortably in fp32/bf16 range and matches jax softmax to
fp32 noise.
"""

import numpy as np
import ml_dtypes

try:  # persistent jit cache: skips ~1s/call of XLA->walrus recompile
    import jax

    jax.config.update("jax_compilation_cache_dir", "/tmp/jax_cc_cache")
    jax.config.update("jax_persistent_cache_min_entry_size_bytes", 0)
    jax.config.update("jax_persistent_cache_min_compile_time_secs", 0.0)
except Exception:
    pass

import concourse.bass as bass
import concourse.mybir as mybir
import concourse.tile as tile
from concourse.masks import make_identity

F32 = mybir.dt.float32
BF16 = mybir.dt.bfloat16
AF = mybir.ActivationFunctionType

S = 2048  # sequence length (both q and kv)
D = 1024  # model dim
NB = 2  # batches
HL = 2  # local heads per core
HD = 64  # head dim
CS = HL * HD  # local channel slice = 128
P = 128
DC = D // P  # 8 d-chunks
NSC = S // P  # 16 s-chunks
NQT = S // P  # 16 q-tiles
XROWS = NB * 2 * S  # 8192 rows of X = [xh0; xh1; xc0; xc1]
XSL = XROWS // 8  # 1024 rows shipped per core
OROWS = NB * S  # 4096 rows of stacked partial output
OSL = OROWS // 8  # 512 rows of final output per core

GROUPS = [list(range(8))]


def build_program(iters=1):
    nc = bass.Bass("TRN2", target_bir_lowering=False, debug=False, num_devices=8)

    # all bf16 payload packed into one tensor: [x-slice | wq | wk | wv | wo^T]
    xw = nc.dram_tensor("xw", [XSL, D + 4 * CS], BF16, kind="ExternalInput").ap()
    bc = nc.dram_tensor("bc", [P, 2], F32, kind="ExternalInput").ap()  # bq, bk cols
    bvr = nc.dram_tensor("bvr", [1, CS], F32, kind="ExternalInput").ap()
    out_s = nc.dram_tensor("out_s", [OSL, D], BF16, kind="ExternalOutput").ap()

    with tile.TileContext(nc) as tc:
        with (
            tc.tile_pool(name="dram", bufs=1, space="DRAM") as dram,
            tc.tile_pool(name="persist", bufs=1) as pp,
            tc.tile_pool(name="ptp", bufs=3) as ptp,
            tc.tile_pool(name="osb", bufs=2) as osb,
            tc.tile_pool(name="outp", bufs=2) as outp,
            tc.tile_pool(name="smalls", bufs=2) as smalls,
            tc.tile_pool(name="ps", bufs=3, space="PSUM") as ps,
        ):
            # ---- DRAM bounces for the collectives ----
            x_bounce = dram.tile([XSL, D], BF16)
            Xg = dram.tile([XROWS, D], BF16)
            out_part = dram.tile([OROWS, D], BF16)
            out_red = dram.tile([OSL, D], BF16)

            # DRAM->DRAM DMA is not safe in this environment (NRT exec-unit
            # crash); bounce the input through SBUF instead.
            for i in range(XSL // P):
                xtmp = outp.tile([P, D], BF16, tag="xtmp", name="xtmp")
                nc.sync.dma_start(xtmp, xw[i * P : (i + 1) * P, 0:D])
                nc.sync.dma_start(x_bounce[i * P : (i + 1) * P, :], xtmp)
            nc.gpsimd.collective_compute(
                "AllGather",
                mybir.AluOpType.bypass,
                replica_groups=GROUPS,
                ins=[x_bounce.opt()],
                outs=[Xg.opt()],
            )

            # ---- persistent SBUF state ----
            wq_sb = pp.tile([P, DC, CS], BF16)
            wk_sb = pp.tile([P, DC, CS], BF16)
            wv_sb = pp.tile([P, DC, CS], BF16)
            wo_sb = pp.tile([P, D], BF16)
            bqc = pp.tile([P, 1], F32)
            bkc = pp.tile([P, 1], F32)
            bv_row = pp.tile([1, CS], F32)
            ones1 = pp.tile([1, P], F32)
            bv_bc = pp.tile([P, CS], F32)
            ident = pp.tile([P, P], BF16)

            W0 = D
            for dc in range(DC):
                rs = slice(dc * P, (dc + 1) * P)
                nc.sync.dma_start(wq_sb[:, dc, :], xw[rs, W0 : W0 + CS])
                nc.sync.dma_start(wk_sb[:, dc, :], xw[rs, W0 + CS : W0 + 2 * CS])
                nc.sync.dma_start(wv_sb[:, dc, :], xw[rs, W0 + 2 * CS : W0 + 3 * CS])
            # wo is shipped transposed; XBAR-transpose it back to [c, d] layout
            nc.sync.dma_start_transpose(wo_sb, xw[:, W0 + 3 * CS : W0 + 4 * CS])
            nc.sync.dma_start(bqc, bc[:, 0:1])
            nc.sync.dma_start(bkc, bc[:, 1:2])
            nc.sync.dma_start(bv_row, bvr)
            make_identity(nc, ident)
            nc.vector.memset(ones1, 1.0)

            # broadcast bv across partitions: [1,128] -> [128,128] via fp32 matmul
            psb = ps.tile([P, CS], F32, tag="o", bufs=2, name="psb")
            nc.tensor.matmul(psb, ones1, bv_row, start=True, stop=True)
            nc.vector.tensor_copy(bv_bc, psb)

            def one_batch(b):
                xhT = pp.tile([P, DC, S], BF16, tag="xhT", name="xhT")
                xcT = pp.tile([P, DC, S], BF16, tag="xcT", name="xcT")
                QT = pp.tile([P, S], BF16, tag="QT", name="QT")
                KT = pp.tile([P, S], BF16, tag="KT", name="KT")
                Vp = pp.tile([P, NSC, HL, HD + 1], BF16, tag="Vp", name="Vp")
                OT = pp.tile([P, S], BF16, tag="OT", name="OT")
                # ones column for the PV sums trick; V writes only fill [..., 0:64]
                nc.vector.memset(Vp, 1.0)

                xh_rows = b * S  # rows of xh[b] in Xg
                xc_rows = 2 * S + b * S  # rows of xc[b] in Xg

                # ---- transposed input loads (XBAR dma transpose, bf16) ----
                SG = 512
                for g in range(S // SG):
                    sl = slice(g * SG, (g + 1) * SG)
                    gl = slice(xc_rows + g * SG, xc_rows + (g + 1) * SG)
                    for dc in range(DC):
                        nc.sync.dma_start_transpose(
                            xcT[:, dc, sl], Xg[gl, dc * P : (dc + 1) * P]
                        )
                for g in range(S // SG):
                    sl = slice(g * SG, (g + 1) * SG)
                    gl = slice(xh_rows + g * SG, xh_rows + (g + 1) * SG)
                    for dc in range(DC):
                        nc.sync.dma_start_transpose(
                            xhT[:, dc, sl], Xg[gl, dc * P : (dc + 1) * P]
                        )

                # ---- projections ----
                def proj_kv(sc):
                    ss = sc * P
                    psk = ps.tile([P, P], F32, tag="o", bufs=2, name="psk")
                    for dc in range(DC):
                        nc.tensor.matmul(
                            psk,
                            wk_sb[:, dc, :],
                            xcT[:, dc, ss : ss + P],
                            start=(dc == 0),
                            stop=(dc == DC - 1),
                        )
                    nc.vector.tensor_scalar_add(KT[:, ss : ss + P], psk, bkc)
                    psv = ps.tile([P, CS], F32, tag="o", bufs=2, name="psv")
                    for dc in range(DC):
                        nc.tensor.matmul(
                            psv,
                            xcT[:, dc, ss : ss + P],
                            wv_sb[:, dc, :],
                            start=(dc == 0),
                            stop=(dc == DC - 1),
                        )
                    for h in range(HL):
                        nc.vector.tensor_add(
                            Vp[:, sc, h, 0:HD],
                            psv[:, h * HD : (h + 1) * HD],
                            bv_bc[:, h * HD : (h + 1) * HD],
                        )

                def proj_q(sc):
                    ss = sc * P
                    psq = ps.tile([P, P], F32, tag="o", bufs=2, name="psq")
                    for dc in range(DC):
                        nc.tensor.matmul(
                            psq,
                            wq_sb[:, dc, :],
                            xhT[:, dc, ss : ss + P],
                            start=(dc == 0),
                            stop=(dc == DC - 1),
                        )
                    nc.vector.tensor_scalar_add(QT[:, ss : ss + P], psq, bqc)

                for sc in range(NSC):
                    proj_kv(sc)

                # ---- attention + output projection, per q-tile ----
                scale = 1.0 / float(np.sqrt(HD))
                NHALF = NSC // 2  # 8 k-chunks per half
                for qt in range(NQT):
                    qs = qt * P
                    proj_q(qt)
                    ps_o = [
                        ps.tile([P, HD + 1], F32, tag="o", bufs=2, name="ps_o")
                        for _ in range(2)
                    ]
                    for half in range(2):
                        psc = [
                            ps.tile([P, NHALF, P], F32, tag="s", name="psc")
                            for _ in range(2)
                        ]
                        for j in range(NHALF):
                            ks = (half * NHALF + j) * P
                            for hh in range(2):
                                e0 = hh * HD
                                nc.tensor.matmul(
                                    psc[hh][:, j, :],
                                    KT[e0 : e0 + HD, ks : ks + P],
                                    QT[e0 : e0 + HD, qs : qs + P],
                                    start=True,
                                    stop=True,
                                )
                        pt = [
                            ptp.tile([P, NHALF, P], BF16, tag="pt", name="pt")
                            for _ in range(2)
                        ]
                        for hh in range(2):
                            nc.scalar.activation(pt[hh], psc[hh], AF.Exp, scale=scale)
                        for j in range(NHALF):
                            kc = half * NHALF + j
                            for hh in range(2):
                                nc.tensor.matmul(
                                    ps_o[hh],
                                    pt[hh][:, j, :],
                                    Vp[:, kc, hh, :],
                                    start=(kc == 0),
                                    stop=(kc == NSC - 1),
                                )
                    recip = smalls.tile([P, 2], F32, tag="recip")
                    for hh in range(2):
                        nc.vector.reciprocal(
                            recip[:, hh : hh + 1], ps_o[hh][:, HD : HD + 1]
                        )
                    for hh in range(2):
                        e0 = hh * HD
                        o_sb = osb.tile([P, HD], BF16, tag="o_sb")
                        nc.vector.tensor_scalar_mul(
                            o_sb, ps_o[hh][:, 0:HD], recip[:, hh : hh + 1]
                        )
                        ps_ot = ps.tile([HD, P], BF16, tag="o", bufs=2, name="ps_ot")
                        nc.tensor.transpose(ps_ot, o_sb, ident)
                        nc.vector.tensor_copy(OT[e0 : e0 + HD, qs : qs + P], ps_ot)
                    out_sb = outp.tile([P, D], BF16, tag="out_sb")
                    for nj in range(2):
                        ps3 = ps.tile([P, D // 2], F32, tag="o", bufs=2, name="ps3")
                        nc.tensor.matmul(
                            ps3,
                            OT[:, qs : qs + P],
                            wo_sb[:, nj * (D // 2) : (nj + 1) * (D // 2)],
                            start=True,
                            stop=True,
                        )
                        nc.vector.tensor_copy(
                            out_sb[:, nj * (D // 2) : (nj + 1) * (D // 2)], ps3
                        )
                    nc.sync.dma_start(
                        out_part[b * S + qs : b * S + qs + P, :], out_sb
                    )

            def one_pass():
                for b in range(NB):
                    one_batch(b)

                # ---- on-device all-reduce of the wo partials (bf16 CCE add) ----
                nc.gpsimd.collective_compute(
                    "ReduceScatter",
                    mybir.AluOpType.add,
                    replica_groups=GROUPS,
                    ins=[out_part.opt()],
                    outs=[out_red.opt()],
                )
                # bounce DRAM->SBUF->DRAM (direct DRAM->DRAM DMA crashes NRT here)
                for i in range(OSL // P):
                    finb = outp.tile([P, D], BF16, tag="finb", name="finb")
                    nc.sync.dma_start(finb, out_red[i * P : (i + 1) * P, :])
                    nc.sync.dma_start(out_s[i * P : (i + 1) * P, :], finb)

            for _it in range(iters):
                one_pass()

    _split_matmul_waits(nc)
    return nc


_WAIT_EXEMPT = (
    mybir.InstAllEngineBarrier,
    mybir.InstEventSemaphore,
    mybir.InstNoOp,
)


def _split_matmul_waits(nc):
    """This walrus' ISA structs encode only ONE sync wait per instruction;
    Tile can emit 2+ (e.g. fresh-DMA input + psum-slot WAR). Hoist all but
    one wait onto engine no-ops inserted just before the instruction."""
    n_id = 0
    for blk in nc.m.functions[0].blocks:
        new_insts = []
        for inst in blk.instructions:
            si = getattr(inst, "sync_info", None)
            if (
                not isinstance(inst, _WAIT_EXEMPT)
                and si is not None
                and si.on_wait
                and len(si.on_wait) > 1
            ):
                waits = list(si.on_wait)
                for w in waits[:-1]:
                    n_id += 1
                    new_insts.append(
                        mybir.InstNoOp(
                            name=f"I-waitsplit-{n_id}",
                            engine=inst.engine,
                            sync_info=mybir.SyncInfo(on_wait=[w], on_update=[]),
                        )
                    )
                inst.sync_info = mybir.SyncInfo(
                    on_wait=[waits[-1]], on_update=list(si.on_update or [])
                )
            new_insts.append(inst)
        blk.instructions[:] = new_insts
    return nc


_CACHE = {}


def _get_program():
    if "nc" not in _CACHE:
        _CACHE["nc"] = build_program()
    return _CACHE["nc"]


def make_in_maps(inputs):
    bf16 = ml_dtypes.bfloat16
    hid = np.asarray(inputs["hidden_states"])
    ctx = np.asarray(inputs["context"])
    # X = [xh0; xh1; xc0; xc1] : [8192, 1024]
    X = np.concatenate([hid[0], hid[1], ctx[0], ctx[1]], axis=0).astype(bf16)
    wq_b = np.asarray(inputs["wq"]).astype(bf16)
    wk_b = np.asarray(inputs["wk"]).astype(bf16)
    wv_b = np.asarray(inputs["wv"]).astype(bf16)
    wo_b = np.asarray(inputs["wo"]).astype(bf16)
    bq_f = np.asarray(inputs["bq"], np.float32)
    bk_f = np.asarray(inputs["bk"], np.float32)
    bv_f = np.asarray(inputs["bv"], np.float32)
    in_maps = []
    for cid in range(8):
        cs = slice(cid * CS, (cid + 1) * CS)
        xw = np.concatenate(
            [
                X[cid * XSL : (cid + 1) * XSL],
                wq_b[:, cs],
                wk_b[:, cs],
                wv_b[:, cs],
                np.ascontiguousarray(wo_b[cs, :].T),
            ],
            axis=1,
        )
        in_maps.append(
            {
                "xw": np.ascontiguousarray(xw),
                "bc": np.ascontiguousarray(np.stack([bq_f[cs], bk_f[cs]], axis=1)),
                "bvr": np.ascontiguousarray(bv_f[cs][None, :]),
            }
        )
    return in_maps


def assemble(results, inputs):
    out = np.concatenate(
        [np.asarray(results[c]["out_s"], np.float32) for c in range(8)], axis=0
    ).reshape(NB, S, D)
    out += np.asarray(inputs["bo"], np.float32)[None, None, :]
    return out


def kernel(**inputs):
    from concourse.bass_utils import run_bass_kernel_spmd

    nc = _get_program()
    in_maps = make_in_maps(inputs)
    res = run_bass_kernel_spmd(nc, in_maps, list(range(8)))
    return assemble(res.results, inputs)


# revision 15
# speedup vs baseline: 1.0279x; 1.0066x over previous
"""Cross-attention Trainium2 Bass kernel, 8-core SPMD.

Problem (hardcoded): B=2, SQ=SKV=2048, DIM=1024, H=16, hd=64, fp32 I/O.

Sharding: 8 cores = 8 head-pairs (2 heads / 128 cols each); every core
processes BOTH batches. Tensor-parallel on wq/wk/wv columns and wo rows.
Unlike the earlier 2x4 layout this ships each weight slice exactly once
and never duplicates activations: each core uploads a distinct 1/8 slice
of X=[xh0;xh1;xc0;xc1] ([8192,1024] bf16) and an on-device AllGather
rebuilds the full X. The wo-row all-reduce is an on-device bf16
ReduceScatter over all 8 cores, so each core downloads only its
[512,1024] slice of the final output (bf16); the host just concatenates
and adds bo. This cuts host<->device traffic ~5x vs shipping full
duplicated inputs and full fp32 partial outputs, which dominates
wall-clock through the axon tunnel.

Per-core dataflow (all matmuls bf16 with fp32 PSUM accumulation), per
batch b in {0,1}:
  1. DMA-transpose the bf16 inputs into x^T layout [128d, dc, S] straight
     from the gathered X in DRAM (XBAR path), then project to Q^T/K^T in
     head-on-partition layout [128c, S] (+bias per partition) and V in
     natural layout [128k, kc, h, 65] with a ones column appended so the
     PV matmul also produces softmax denominators.
  2. Flash-style attention per q-tile of 128: scores^T chunks [k128,
     q128] accumulate in PSUM (8 k-chunks per half), one exp on ScalarE
     per (head, half) (PSUM fp32 -> SBUF bf16, scale=1/8 folded in),
     then PV with P^T stationary: psum_o[q, 65] += P^T.T@V'. The two
     heads sit in partition rows 0-63/64-127 so the PE 64-row array
     tiling can run both heads' matmuls concurrently.
  3. Normalize O by the per-partition reciprocal of column 64, small PE
     transpose to O^T, then the wo projection out[q,1024] per q-tile
     into the [4096,1024] f32 partial that feeds the ReduceScatter.
Softmax skips the max-subtraction: scores here are ~N(0,1) (max |s| ~ 8),
so exp stays comfortably in fp32/bf16 range and matches jax softmax to
fp32 noise.
"""

import numpy as np
import ml_dtypes

try:  # persistent jit cache: skips ~1s/call of XLA->walrus recompile
    import jax

    jax.config.update("jax_compilation_cache_dir", "/tmp/jax_cc_cache")
    jax.config.update("jax_persistent_cache_min_entry_size_bytes", 0)
    jax.config.update("jax_persistent_cache_min_compile_time_secs", 0.0)
except Exception:
    pass

import concourse.bass as bass
import concourse.mybir as mybir
import concourse.tile as tile
from concourse.masks import make_identity

F32 = mybir.dt.float32
BF16 = mybir.dt.bfloat16
AF = mybir.ActivationFunctionType

S = 2048  # sequence length (both q and kv)
D = 1024  # model dim
NB = 2  # batches
HL = 2  # local heads per core
HD = 64  # head dim
CS = HL * HD  # local channel slice = 128
P = 128
DC = D // P  # 8 d-chunks
NSC = S // P  # 16 s-chunks
NQT = S // P  # 16 q-tiles
XROWS = NB * 2 * S  # 8192 rows of X = [xh0; xh1; xc0; xc1]
XSL = XROWS // 8  # 1024 rows shipped per core
OROWS = NB * S  # 4096 rows of stacked partial output
OSL = OROWS // 8  # 512 rows of final output per core

GROUPS = [list(range(8))]


def build_program(iters=1, collectives=True):
    nc = bass.Bass("TRN2", target_bir_lowering=False, debug=False, num_devices=8)

    # all bf16 payload packed into one tensor: [x-slice | wq | wk | wv | wo^T]
    xw = nc.dram_tensor("xw", [XSL, D + 4 * CS], BF16, kind="ExternalInput").ap()
    bc = nc.dram_tensor("bc", [P, 2], F32, kind="ExternalInput").ap()  # bq, bk cols
    bvr = nc.dram_tensor("bvr", [1, CS], F32, kind="ExternalInput").ap()
    out_s = nc.dram_tensor("out_s", [OSL, D], BF16, kind="ExternalOutput").ap()

    with tile.TileContext(nc) as tc:
        with (
            tc.tile_pool(name="dram", bufs=1, space="DRAM") as dram,
            tc.tile_pool(name="persist", bufs=1) as pp,
            tc.tile_pool(name="ptp", bufs=3) as ptp,
            tc.tile_pool(name="osb", bufs=2) as osb,
            tc.tile_pool(name="outp", bufs=2) as outp,
            tc.tile_pool(name="smalls", bufs=2) as smalls,
            tc.tile_pool(name="ps", bufs=3, space="PSUM") as ps,
        ):
            # ---- DRAM bounces for the collectives ----
            x_bounce = dram.tile([XSL, D], BF16)
            Xg = dram.tile([XROWS, D], BF16)
            out_part = dram.tile([OROWS, D], BF16)
            out_red = dram.tile([OSL, D], BF16)

            # DRAM->DRAM DMA is not safe in this environment (NRT exec-unit
            # crash); bounce the input through SBUF instead.
            for i in range(XSL // P):
                xtmp = outp.tile([P, D], BF16, tag="xtmp", name="xtmp")
                nc.sync.dma_start(xtmp, xw[i * P : (i + 1) * P, 0:D])
                nc.sync.dma_start(x_bounce[i * P : (i + 1) * P, :], xtmp)
            if collectives:
                nc.gpsimd.collective_compute(
                    "AllGather",
                    mybir.AluOpType.bypass,
                    replica_groups=GROUPS,
                    ins=[x_bounce.opt()],
                    outs=[Xg.opt()],
                )

            # ---- persistent SBUF state ----
            wq_sb = pp.tile([P, DC, CS], BF16)
            wk_sb = pp.tile([P, DC, CS], BF16)
            wv_sb = pp.tile([P, DC, CS], BF16)
            wo_sb = pp.tile([P, D], BF16)
            bqc = pp.tile([P, 1], F32)
            bkc = pp.tile([P, 1], F32)
            bv_row = pp.tile([1, CS], F32)
            ones1 = pp.tile([1, P], F32)
            bv_bc = pp.tile([P, CS], F32)
            ident = pp.tile([P, P], BF16)

            W0 = D
            for dc in range(DC):
                rs = slice(dc * P, (dc + 1) * P)
                nc.sync.dma_start(wq_sb[:, dc, :], xw[rs, W0 : W0 + CS])
                nc.sync.dma_start(wk_sb[:, dc, :], xw[rs, W0 + CS : W0 + 2 * CS])
                nc.sync.dma_start(wv_sb[:, dc, :], xw[rs, W0 + 2 * CS : W0 + 3 * CS])
            # wo is shipped transposed; XBAR-transpose it back to [c, d] layout
            nc.sync.dma_start_transpose(wo_sb, xw[:, W0 + 3 * CS : W0 + 4 * CS])
            nc.sync.dma_start(bqc, bc[:, 0:1])
            nc.sync.dma_start(bkc, bc[:, 1:2])
            nc.sync.dma_start(bv_row, bvr)
            make_identity(nc, ident)
            nc.vector.memset(ones1, 1.0)

            # broadcast bv across partitions: [1,128] -> [128,128] via fp32 matmul
            psb = ps.tile([P, CS], F32, tag="o", bufs=2, name="psb")
            nc.tensor.matmul(psb, ones1, bv_row, start=True, stop=True)
            nc.vector.tensor_copy(bv_bc, psb)

            def one_batch(b):
                xhT = pp.tile([P, DC, S], BF16, tag="xhT", name="xhT")
                xcT = pp.tile([P, DC, S], BF16, tag="xcT", name="xcT")
                QT = pp.tile([P, S], BF16, tag="QT", name="QT")
                KT = pp.tile([P, S], BF16, tag="KT", name="KT")
                Vp = pp.tile([P, NSC, HL, HD + 1], BF16, tag="Vp", name="Vp")
                OT = pp.tile([P, S], BF16, tag="OT", name="OT")
                # ones column for the PV sums trick; V writes only fill [..., 0:64]
                nc.vector.memset(Vp, 1.0)

                xh_rows = b * S  # rows of xh[b] in Xg
                xc_rows = 2 * S + b * S  # rows of xc[b] in Xg

                # ---- transposed input loads (XBAR dma transpose, bf16) ----
                SG = 512
                for g in range(S // SG):
                    sl = slice(g * SG, (g + 1) * SG)
                    gl = slice(xc_rows + g * SG, xc_rows + (g + 1) * SG)
                    for dc in range(DC):
                        nc.sync.dma_start_transpose(
                            xcT[:, dc, sl], Xg[gl, dc * P : (dc + 1) * P]
                        )
                for g in range(S // SG):
                    sl = slice(g * SG, (g + 1) * SG)
                    gl = slice(xh_rows + g * SG, xh_rows + (g + 1) * SG)
                    for dc in range(DC):
                        nc.sync.dma_start_transpose(
                            xhT[:, dc, sl], Xg[gl, dc * P : (dc + 1) * P]
                        )

                # ---- projections ----
                def proj_kv(sc):
                    ss = sc * P
                    psk = ps.tile([P, P], F32, tag="o", bufs=2, name="psk")
                    for dc in range(DC):
                        nc.tensor.matmul(
                            psk,
                            wk_sb[:, dc, :],
                            xcT[:, dc, ss : ss + P],
                            start=(dc == 0),
                            stop=(dc == DC - 1),
                        )
                    nc.vector.tensor_scalar_add(KT[:, ss : ss + P], psk, bkc)
                    psv = ps.tile([P, CS], F32, tag="o", bufs=2, name="psv")
                    for dc in range(DC):
                        nc.tensor.matmul(
                            psv,
                            xcT[:, dc, ss : ss + P],
                            wv_sb[:, dc, :],
                            start=(dc == 0),
                            stop=(dc == DC - 1),
                        )
                    for h in range(HL):
                        nc.vector.tensor_add(
                            Vp[:, sc, h, 0:HD],
                            psv[:, h * HD : (h + 1) * HD],
                            bv_bc[:, h * HD : (h + 1) * HD],
                        )

                def proj_q(sc):
                    ss = sc * P
                    psq = ps.tile([P, P], F32, tag="o", bufs=2, name="psq")
                    for dc in range(DC):
                        nc.tensor.matmul(
                            psq,
                            wq_sb[:, dc, :],
                            xhT[:, dc, ss : ss + P],
                            start=(dc == 0),
                            stop=(dc == DC - 1),
                        )
                    nc.vector.tensor_scalar_add(QT[:, ss : ss + P], psq, bqc)

                for sc in range(NSC):
                    proj_kv(sc)

                # ---- attention + output projection, per q-tile ----
                scale = 1.0 / float(np.sqrt(HD))
                NHALF = NSC // 2  # 8 k-chunks per half
                for qt in range(NQT):
                    qs = qt * P
                    proj_q(qt)
                    ps_o = [
                        ps.tile([P, HD + 1], F32, tag="o", bufs=2, name="ps_o")
                        for _ in range(2)
                    ]
                    for half in range(2):
                        psc = [
                            ps.tile([P, NHALF, P], F32, tag="s", name="psc")
                            for _ in range(2)
                        ]
                        for j in range(NHALF):
                            ks = (half * NHALF + j) * P
                            for hh in range(2):
                                e0 = hh * HD
                                nc.tensor.matmul(
                                    psc[hh][:, j, :],
                                    KT[e0 : e0 + HD, ks : ks + P],
                                    QT[e0 : e0 + HD, qs : qs + P],
                                    start=True,
                                    stop=True,
                                )
                        pt = [
                            ptp.tile([P, NHALF, P], BF16, tag="pt", name="pt")
                            for _ in range(2)
                        ]
                        for hh in range(2):
                            nc.scalar.activation(pt[hh], psc[hh], AF.Exp, scale=scale)
                        for j in range(NHALF):
                            kc = half * NHALF + j
                            for hh in range(2):
                                nc.tensor.matmul(
                                    ps_o[hh],
                                    pt[hh][:, j, :],
                                    Vp[:, kc, hh, :],
                                    start=(kc == 0),
                                    stop=(kc == NSC - 1),
                                )
                    recip = smalls.tile([P, 2], F32, tag="recip")
                    for hh in range(2):
                        nc.vector.reciprocal(
                            recip[:, hh : hh + 1], ps_o[hh][:, HD : HD + 1]
                        )
                    for hh in range(2):
                        e0 = hh * HD
                        o_sb = osb.tile([P, HD], BF16, tag="o_sb")
                        nc.vector.tensor_scalar_mul(
                            o_sb, ps_o[hh][:, 0:HD], recip[:, hh : hh + 1]
                        )
                        ps_ot = ps.tile([HD, P], BF16, tag="o", bufs=2, name="ps_ot")
                        nc.tensor.transpose(ps_ot, o_sb, ident)
                        nc.vector.tensor_copy(OT[e0 : e0 + HD, qs : qs + P], ps_ot)
                    out_sb = outp.tile([P, D], BF16, tag="out_sb")
                    for nj in range(2):
                        ps3 = ps.tile([P, D // 2], F32, tag="o", bufs=2, name="ps3")
                        nc.tensor.matmul(
                            ps3,
                            OT[:, qs : qs + P],
                            wo_sb[:, nj * (D // 2) : (nj + 1) * (D // 2)],
                            start=True,
                            stop=True,
                        )
                        nc.vector.tensor_copy(
                            out_sb[:, nj * (D // 2) : (nj + 1) * (D // 2)], ps3
                        )
                    nc.sync.dma_start(
                        out_part[b * S + qs : b * S + qs + P, :], out_sb
                    )

            def one_pass():
                for b in range(NB):
                    one_batch(b)

                # ---- on-device all-reduce of the wo partials (bf16 CCE add) ----
                if collectives:
                    nc.gpsimd.collective_compute(
                        "ReduceScatter",
                        mybir.AluOpType.add,
                        replica_groups=GROUPS,
                        ins=[out_part.opt()],
                        outs=[out_red.opt()],
                    )
                # bounce DRAM->SBUF->DRAM (direct DRAM->DRAM DMA crashes NRT here)
                for i in range(OSL // P):
                    finb = outp.tile([P, D], BF16, tag="finb", name="finb")
                    nc.sync.dma_start(finb, out_red[i * P : (i + 1) * P, :])
                    nc.sync.dma_start(out_s[i * P : (i + 1) * P, :], finb)

            for _it in range(iters):
                one_pass()

    _split_matmul_waits(nc)
    return nc


_WAIT_EXEMPT = (
    mybir.InstAllEngineBarrier,
    mybir.InstEventSemaphore,
    mybir.InstNoOp,
)


def _split_matmul_waits(nc):
    """This walrus' ISA structs encode only ONE sync wait per instruction;
    Tile can emit 2+ (e.g. fresh-DMA input + psum-slot WAR). Hoist all but
    one wait onto engine no-ops inserted just before the instruction."""
    n_id = 0
    for blk in nc.m.functions[0].blocks:
        new_insts = []
        for inst in blk.instructions:
            si = getattr(inst, "sync_info", None)
            if (
                not isinstance(inst, _WAIT_EXEMPT)
                and si is not None
                and si.on_wait
                and len(si.on_wait) > 1
            ):
                waits = list(si.on_wait)
                for w in waits[:-1]:
                    n_id += 1
                    new_insts.append(
                        mybir.InstNoOp(
                            name=f"I-waitsplit-{n_id}",
                            engine=inst.engine,
                            sync_info=mybir.SyncInfo(on_wait=[w], on_update=[]),
                        )
                    )
                inst.sync_info = mybir.SyncInfo(
                    on_wait=[waits[-1]], on_update=list(si.on_update or [])
                )
            new_insts.append(inst)
        blk.instructions[:] = new_insts
    return nc


_CACHE = {}


def _get_program():
    if "nc" not in _CACHE:
        _CACHE["nc"] = build_program()
    return _CACHE["nc"]


def make_in_maps(inputs):
    bf16 = ml_dtypes.bfloat16
    hid = np.asarray(inputs["hidden_states"])
    ctx = np.asarray(inputs["context"])
    # X = [xh0; xh1; xc0; xc1] : [8192, 1024]
    X = np.concatenate([hid[0], hid[1], ctx[0], ctx[1]], axis=0).astype(bf16)
    wq_b = np.asarray(inputs["wq"]).astype(bf16)
    wk_b = np.asarray(inputs["wk"]).astype(bf16)
    wv_b = np.asarray(inputs["wv"]).astype(bf16)
    wo_b = np.asarray(inputs["wo"]).astype(bf16)
    bq_f = np.asarray(inputs["bq"], np.float32)
    bk_f = np.asarray(inputs["bk"], np.float32)
    bv_f = np.asarray(inputs["bv"], np.float32)
    in_maps = []
    for cid in range(8):
        cs = slice(cid * CS, (cid + 1) * CS)
        xw = np.concatenate(
            [
                X[cid * XSL : (cid + 1) * XSL],
                wq_b[:, cs],
                wk_b[:, cs],
                wv_b[:, cs],
                np.ascontiguousarray(wo_b[cs, :].T),
            ],
            axis=1,
        )
        in_maps.append(
            {
                "xw": np.ascontiguousarray(xw),
                "bc": np.ascontiguousarray(np.stack([bq_f[cs], bk_f[cs]], axis=1)),
                "bvr": np.ascontiguousarray(bv_f[cs][None, :]),
            }
        )
    return in_maps


def assemble(results, inputs):
    out = np.concatenate(
        [np.asarray(results[c]["out_s"], np.float32) for c in range(8)], axis=0
    ).reshape(NB, S, D)
    out += np.asarray(inputs["bo"], np.float32)[None, None, :]
    return out


def kernel(**inputs):
    import time

    from concourse.bass_utils import run_bass_kernel_spmd

    nc = _get_program()
    in_maps = make_in_maps(inputs)
    # one retry with backoff: the axon-tunneled device occasionally wedges
    # transiently (UNAVAILABLE/INTERNAL) and recovers within ~1-2 min
    try:
        res = run_bass_kernel_spmd(nc, in_maps, list(range(8)))
    except Exception:
        time.sleep(90)
        res = run_bass_kernel_spmd(nc, in_maps, list(range(8)))
    return assemble(res.results, inputs)
